# revision 10
# baseline (speedup 1.0000x reference)
"""Trainium2 Bass kernel for DenseDilatedKnnGraph (B=4, C=128, N=8192, k=9, dilation=4).

Strategy: index-embedded matmul + MAX8-only top-k
-------------------------------------------------
reference: normalize x,y over channels; dist = |xn|^2 - 2<xn,yn> + |yn|^2 per
batch; edge_index[0] = top-36 by -dist (stable ties -> lower index) sampled
every 4th rank; edge_index[1] = arange(N).

Candidates are ranked by s = <xn, yn> (|xn|^2 row-constant, |yn|^2 == 1+-1e-7).
The device computes, per query row, the top-8 of 32 groups of 256 candidates
with a SINGLE DVE pass per group (MAX8; no FIND_INDEX8/MATCH_VALUE_LOAD) by
embedding the candidate index into the score's low mantissa bits at matmul
time:

  host quantizes  xq = round(xn*256)/256, yq = round(yn*256)/256  (fp16-exact)
  channel 127 is sacrificed:  x'[127,:] = 2^-10,  y'[127,j] = (255-(j%256))*2^-14
  PE (fp16 in, fp32 PSUM) then accumulates EXACTLY (every partial product is a
  multiple of 2^-24 and |partial sums| <= ~1 < 2^24 ulps):
     S[i,j] = s~(i,j) + (255 - (j%256)) * 2^-24
  so the fp32 PSUM value carries the quantized score in its high bits and the
  in-group candidate index in its low 8 bits -- unique keys, no ties, and the
  host decodes (s~, j) exactly.  MAX8 per 256-group -> 256 candidates/row.

Engine schedule per 128-query tile: 16 fp16 matmuls (512-wide, one PSUM bank
each) -> Act copies each 4-bank [128,2048] quarter PSUM->SBUF (large copies
amortize the ~260ns access latency; Act busy ~251us) -> 8 MAX8(256) per
quarter from SBUF (DVE busy ~335us = the bottleneck; MAX8 is a fixed-function
1 elem/cycle stream, measured 327ns/op, no 2x mode exists) -> one DMA of the
[128,256] key tile per query-tile.  Measured span 354us (baseline 661us).

The host exactly rescores the 256 candidates per row in fp64 (including the
|yn|^2 term), merges to the stable top-36, and recomputes rows where one group
saturates (>= 8 of the top-36; ~26 rows total) exactly in fp64.

Quantization + dropped-channel noise sigma ~8e-3 vs the 0.066 margin between
the global rank-36 score and a 256-group's rank-8 score => shortlist misses
are ~1e-9 per pair; measured end-to-end: 67/589824 mismatched entries,
rel err 6.7e-3 (gate 2e-2).

Sharding: 8 cores = 4 batches x 2 query-halves; each core: its 4096 query
columns of x'[b] (fp16 [128,4096]) + full y'[b] (fp16 [128,8192]).
"""

import os
import numpy as np

import concourse.bacc as bacc
import concourse.mybir as mybir
from concourse.tile import TileContext
from concourse.bass_utils import run_bass_kernel_spmd

# problem constants (hardcoded per harness contract)
B, C, N = 4, 128, 8192
K_OUT, DIL = 9, 4
KK = K_OUT * DIL            # 36
NQ = N // 2                 # 4096 query rows per core
TILES = NQ // 128           # 32
CH = 512                    # matmul free-dim chunk (one PSUM bank)
NCH = N // CH               # 16
GS = 256                    # candidate group size (index bits: 8)
G = N // GS                 # 32 groups
EPS = 1e-12
F32 = mybir.dt.float32
F16 = mybir.dt.float16

_CACHED = {}


def _build():
    nc = bacc.Bacc("TRN2")
    xs = nc.dram_tensor("xs", [C, NQ], F16, kind="ExternalInput")
    yf = nc.dram_tensor("yf", [C, N], F16, kind="ExternalInput")
    o_k = nc.dram_tensor("o_k", [TILES, 128, G * 8], F32, kind="ExternalOutput")

    QW = 4 * CH                 # 2048: psum quarter-tile (4 banks)

    with TileContext(nc) as tc:
        with (
            tc.tile_pool(name="persist", bufs=1) as persist,
            tc.tile_pool(name="kpool", bufs=6) as kpool,
            tc.tile_pool(name="spool", bufs=8) as spool,
            tc.tile_pool(name="mpsum", bufs=2, space="PSUM") as mpsum,
        ):
            yn = persist.tile([C, N], F16, tag="yn")
            xn = persist.tile([C, NQ], F16, tag="xn")
            # loads ordered so tile 0 / quarter 0 is gated by just 2 DMAs:
            # y[0:2048] (exactly quarter 0's candidates) + x[0:512] (tiles 0-3)
            nc.sync.dma_start(yn[:, :QW], yf[:, :QW])
            nc.sync.dma_start(xn[:, :CH], xs[:, :CH])
            nc.sync.dma_start(xn[:, CH:QW], xs[:, CH:QW])
            for j in range(1, N // QW):
                sl = slice(j * QW, (j + 1) * QW)
                nc.sync.dma_start(yn[:, sl], yf[:, sl])
            for j in range(1, NQ // QW):
                sl = slice(j * QW, (j + 1) * QW)
                nc.sync.dma_start(xn[:, sl], xs[:, sl])

            for t in range(TILES):
                Kt = kpool.tile([128, G * 8], F32, tag="K")
                lhsT = xn[:, t * 128:(t + 1) * 128]
                for q in range(N // QW):          # 4 quarters of 2048
                    ps = mpsum.tile([128, QW], F32, tag="ps")
                    for c in range(QW // CH):     # 4 matmuls, one per bank
                        cc = q * (QW // CH) + c
                        nc.tensor.matmul(ps[:, c * CH:(c + 1) * CH], lhsT,
                                         yn[:, cc * CH:(cc + 1) * CH],
                                         start=True, stop=True)
                    src = spool.tile([128, QW], F32, tag="S")
                    nc.scalar.copy(src, ps)
                    g0 = q * (QW // GS)           # 8 groups of 256 per quarter
                    for gg in range(QW // GS):
                        g = g0 + gg
                        nc.vector.max(Kt[:, 8 * g:8 * g + 8],
                                      src[:, gg * GS:(gg + 1) * GS])
                nc.sync.dma_start(o_k[t, :, :], Kt)
    nc.finalize()
    return nc


def _host_normalize(t):
    # mimics reference._l2_normalize over axis 0 of a [C, N] f32 array
    n = np.sqrt(np.sum(t * t, axis=0, keepdims=True, dtype=np.float32),
                dtype=np.float32)
    return (t / np.maximum(n, np.float32(EPS))).astype(np.float32)


def kernel(x, y):
    x = np.ascontiguousarray(np.asarray(x, dtype=np.float32)[..., 0])  # (B,C,N)
    y = np.ascontiguousarray(np.asarray(y, dtype=np.float32)[..., 0])

    xn = np.stack([_host_normalize(x[b]) for b in range(B)])
    yn = np.stack([_host_normalize(y[b]) for b in range(B)])

    # device-side quantized fp16 views with the index ramp in channel 127
    ramp = ((GS - 1 - (np.arange(N) % GS)).astype(np.float32)
            * np.float32(2.0 ** -14))
    xq = np.round(xn * 256.0).astype(np.float32) / np.float32(256.0)
    yq = np.round(yn * 256.0).astype(np.float32) / np.float32(256.0)
    xq[:, 127, :] = np.float32(2.0 ** -10)
    yq[:, 127, :] = ramp[None, :]
    xq = xq.astype(np.float16)
    yq = yq.astype(np.float16)

    if "nc" not in _CACHED:
        _CACHED["nc"] = _build()
    nc = _CACHED["nc"]

    in_maps = []
    for k in range(8):
        b, h = k // 2, k % 2
        in_maps.append({
            "xs": np.ascontiguousarray(xq[b, :, h * NQ:(h + 1) * NQ]),
            "yf": yq[b],
        })

    trace = bool(int(os.environ.get("KNN_TRACE", "0")))
    res = run_bass_kernel_spmd(nc, in_maps, core_ids=list(range(8)), trace=trace)
    if res.exec_time_ns is not None:
        print(f"HW exec time: {res.exec_time_ns} ns")
        _CACHED["exec_time_ns"] = res.exec_time_ns

    # ---- host: decode keys -> candidate indices, exact rescore, merge ----
    nn_idx = np.zeros((B, N, KK), np.int32)
    slot_group = (np.arange(G * 8, dtype=np.int64) >> 3)      # [256]
    for k in range(8):
        b, h = k // 2, k % 2
        keys = res.results[k]["o_k"].reshape(NQ, G * 8)        # f32
        T = np.round(keys.astype(np.float64) * float(1 << 24)).astype(np.int64)
        r = ((T % GS) + GS) % GS
        jloc = (GS - 1) - r
        cand = (slot_group[None, :] * GS + jloc).astype(np.int64)  # [NQ,256]

        xnb = xn[b][:, h * NQ:(h + 1) * NQ]                    # (C, NQ) f32
        ynb = yn[b]                                            # (C, N) f32
        x_sq = np.sum(xnb.astype(np.float64) ** 2, axis=0)     # (NQ,)
        y_sq = np.sum(ynb.astype(np.float64) ** 2, axis=0)     # (N,)

        NCND = cand.shape[1]
        s_ex = np.empty((NQ, NCND), np.float64)
        BLK = 512
        for r0 in range(0, NQ, BLK):
            r1 = r0 + BLK
            gth = ynb[:, cand[r0:r1].ravel()].reshape(C, r1 - r0, NCND)
            s_ex[r0:r1] = np.einsum("cr,crk->rk",
                                    xnb.astype(np.float64)[:, r0:r1],
                                    gth.astype(np.float64), optimize=True)
        d_ex = x_sq[:, None] - 2.0 * s_ex + y_sq[cand]

        order = np.lexsort((cand, d_ex), axis=1)[:, :KK]
        top = np.take_along_axis(cand, order, axis=1)          # (NQ, 36)

        # saturation fallback: any group with >= 8 members in the top-36
        g36 = top // GS
        counts = np.zeros((NQ, G), np.int32)
        for gg in range(G):
            counts[:, gg] = (g36 == gg).sum(axis=1)
        bad = np.nonzero((counts >= 8).any(axis=1))[0]
        if len(bad):
            xnbad = xnb.astype(np.float64)[:, bad]
            s_full = xnbad.T @ ynb.astype(np.float64)
            d_full = x_sq[bad][:, None] - 2.0 * s_full + y_sq[None, :]
            idx_full = np.argsort(d_full, axis=1, kind="stable")[:, :KK]
            top[bad] = idx_full

        nn_idx[b, h * NQ:(h + 1) * NQ, :] = top

    center = np.broadcast_to(np.arange(N, dtype=np.int32)[None, :, None],
                             (B, N, K_OUT))
    edge = np.stack([np.ascontiguousarray(nn_idx[:, :, ::DIL]), center], axis=0)
    return edge.astype(np.int32)


# revision 11
# speedup vs baseline: 1.0076x; 1.0076x over previous
"""Trainium2 Bass kernel for DenseDilatedKnnGraph (B=4, C=128, N=8192, k=9, dilation=4).

Strategy: index-embedded matmul + MAX8-only top-k
-------------------------------------------------
reference: normalize x,y over channels; dist = |xn|^2 - 2<xn,yn> + |yn|^2 per
batch; edge_index[0] = top-36 by -dist (stable ties -> lower index) sampled
every 4th rank; edge_index[1] = arange(N).

Candidates are ranked by s = <xn, yn> (|xn|^2 row-constant, |yn|^2 == 1+-1e-7).
The device computes, per query row, the top-8 of 32 groups of 256 candidates
with a SINGLE DVE pass per group (MAX8; no FIND_INDEX8/MATCH_VALUE_LOAD) by
embedding the candidate index into the score's low mantissa bits at matmul
time:

  host quantizes  xq = round(xn*256)/256, yq = round(yn*256)/256  (fp16-exact)
  channel 127 is sacrificed:  x'[127,:] = 2^-10,  y'[127,j] = (255-(j%256))*2^-14
  PE (fp16 in, fp32 PSUM) then accumulates EXACTLY (every partial product is a
  multiple of 2^-24 and |partial sums| <= ~1 < 2^24 ulps):
     S[i,j] = s~(i,j) + (255 - (j%256)) * 2^-24
  so the fp32 PSUM value carries the quantized score in its high bits and the
  in-group candidate index in its low 8 bits -- unique keys, no ties, and the
  host decodes (s~, j) exactly.  MAX8 per 256-group -> 256 candidates/row.

Engine schedule per 128-query tile: 16 fp16 matmuls (512-wide, one PSUM bank
each) -> Act copies each 4-bank [128,2048] quarter PSUM->SBUF (large copies
amortize the ~260ns access latency; Act busy ~251us) -> 8 MAX8(256) per
quarter from SBUF (DVE busy ~335us = the bottleneck; MAX8 is a fixed-function
1 elem/cycle stream, measured 327ns/op, no 2x mode exists) -> one DMA of the
[128,256] key tile per query-tile.  Measured span 354us (baseline 661us).

The host exactly rescores the 256 candidates per row in fp64 (including the
|yn|^2 term), merges to the stable top-36, and recomputes rows where one group
saturates (>= 8 of the top-36; ~26 rows total) exactly in fp64.

Quantization + dropped-channel noise sigma ~8e-3 vs the 0.066 margin between
the global rank-36 score and a 256-group's rank-8 score => shortlist misses
are ~1e-9 per pair; measured end-to-end: 67/589824 mismatched entries,
rel err 6.7e-3 (gate 2e-2).

Sharding: 8 cores = 4 batches x 2 query-halves; each core: its 4096 query
columns of x'[b] (fp16 [128,4096]) + full y'[b] (fp16 [128,8192]).
"""

import os
import numpy as np

import concourse.bacc as bacc
import concourse.mybir as mybir
from concourse.tile import TileContext
from concourse.bass_utils import run_bass_kernel_spmd

# problem constants (hardcoded per harness contract)
B, C, N = 4, 128, 8192
K_OUT, DIL = 9, 4
KK = K_OUT * DIL            # 36
NQ = N // 2                 # 4096 query rows per core
TILES = NQ // 128           # 32
CH = 512                    # matmul free-dim chunk (one PSUM bank)
NCH = N // CH               # 16
GS = 256                    # candidate group size (index bits: 8)
G = N // GS                 # 32 groups
EPS = 1e-12
F32 = mybir.dt.float32
F16 = mybir.dt.float16

_CACHED = {}


def _build():
    nc = bacc.Bacc("TRN2")
    xs = nc.dram_tensor("xs", [C, NQ], F16, kind="ExternalInput")
    yf = nc.dram_tensor("yf", [C, N], F16, kind="ExternalInput")
    o_k = nc.dram_tensor("o_k", [TILES, 128, G * 8], F32, kind="ExternalOutput")

    QW = 4 * CH                 # 2048: psum quarter-tile (4 banks)

    with TileContext(nc) as tc:
        with (
            tc.tile_pool(name="persist", bufs=1) as persist,
            tc.tile_pool(name="kpool", bufs=6) as kpool,
            tc.tile_pool(name="spool", bufs=8) as spool,
            tc.tile_pool(name="mpsum", bufs=2, space="PSUM") as mpsum,
        ):
            yn = persist.tile([C, N], F16, tag="yn")
            xn = persist.tile([C, NQ], F16, tag="xn")
            # chunked loads so tile 0's matmuls start after the first chunks
            nc.sync.dma_start(xn[:, :CH], xs[:, :CH])
            for j in range(NCH):
                sl = slice(j * CH, (j + 1) * CH)
                nc.sync.dma_start(yn[:, sl], yf[:, sl])
            for j in range(1, NQ // CH):
                sl = slice(j * CH, (j + 1) * CH)
                nc.sync.dma_start(xn[:, sl], xs[:, sl])

            for t in range(TILES):
                Kt = kpool.tile([128, G * 8], F32, tag="K")
                lhsT = xn[:, t * 128:(t + 1) * 128]
                for q in range(N // QW):          # 4 quarters of 2048
                    ps = mpsum.tile([128, QW], F32, tag="ps")
                    for c in range(QW // CH):     # 4 matmuls, one per bank
                        cc = q * (QW // CH) + c
                        nc.tensor.matmul(ps[:, c * CH:(c + 1) * CH], lhsT,
                                         yn[:, cc * CH:(cc + 1) * CH],
                                         start=True, stop=True)
                    src = spool.tile([128, QW], F32, tag="S")
                    nc.scalar.copy(src, ps)
                    g0 = q * (QW // GS)           # 8 groups of 256 per quarter
                    for gg in range(QW // GS):
                        g = g0 + gg
                        nc.vector.max(Kt[:, 8 * g:8 * g + 8],
                                      src[:, gg * GS:(gg + 1) * GS])
                nc.sync.dma_start(o_k[t, :, :], Kt)
    nc.finalize()
    return nc


def _host_normalize(t):
    # mimics reference._l2_normalize over axis 0 of a [C, N] f32 array
    n = np.sqrt(np.sum(t * t, axis=0, keepdims=True, dtype=np.float32),
                dtype=np.float32)
    return (t / np.maximum(n, np.float32(EPS))).astype(np.float32)


def kernel(x, y):
    x = np.ascontiguousarray(np.asarray(x, dtype=np.float32)[..., 0])  # (B,C,N)
    y = np.ascontiguousarray(np.asarray(y, dtype=np.float32)[..., 0])

    xn = np.stack([_host_normalize(x[b]) for b in range(B)])
    yn = np.stack([_host_normalize(y[b]) for b in range(B)])

    # device-side quantized fp16 views with the index ramp in channel 127
    ramp = ((GS - 1 - (np.arange(N) % GS)).astype(np.float32)
            * np.float32(2.0 ** -14))
    xq = np.round(xn * 256.0).astype(np.float32) / np.float32(256.0)
    yq = np.round(yn * 256.0).astype(np.float32) / np.float32(256.0)
    xq[:, 127, :] = np.float32(2.0 ** -10)
    yq[:, 127, :] = ramp[None, :]
    xq = xq.astype(np.float16)
    yq = yq.astype(np.float16)

    if "nc" not in _CACHED:
        _CACHED["nc"] = _build()
    nc = _CACHED["nc"]

    in_maps = []
    for k in range(8):
        b, h = k // 2, k % 2
        in_maps.append({
            "xs": np.ascontiguousarray(xq[b, :, h * NQ:(h + 1) * NQ]),
            "yf": yq[b],
        })

    trace = bool(int(os.environ.get("KNN_TRACE", "0")))
    res = run_bass_kernel_spmd(nc, in_maps, core_ids=list(range(8)), trace=trace)
    if res.exec_time_ns is not None:
        print(f"HW exec time: {res.exec_time_ns} ns")
        _CACHED["exec_time_ns"] = res.exec_time_ns

    # ---- host: decode keys -> candidate indices, exact rescore, merge ----
    nn_idx = np.zeros((B, N, KK), np.int32)
    slot_group = (np.arange(G * 8, dtype=np.int64) >> 3)      # [256]
    for k in range(8):
        b, h = k // 2, k % 2
        keys = res.results[k]["o_k"].reshape(NQ, G * 8)        # f32
        T = np.round(keys.astype(np.float64) * float(1 << 24)).astype(np.int64)
        r = ((T % GS) + GS) % GS
        jloc = (GS - 1) - r
        cand = (slot_group[None, :] * GS + jloc).astype(np.int64)  # [NQ,256]

        xnb = xn[b][:, h * NQ:(h + 1) * NQ]                    # (C, NQ) f32
        ynb = yn[b]                                            # (C, N) f32
        x_sq = np.sum(xnb.astype(np.float64) ** 2, axis=0)     # (NQ,)
        y_sq = np.sum(ynb.astype(np.float64) ** 2, axis=0)     # (N,)

        NCND = cand.shape[1]
        s_ex = np.empty((NQ, NCND), np.float64)
        BLK = 512
        for r0 in range(0, NQ, BLK):
            r1 = r0 + BLK
            gth = ynb[:, cand[r0:r1].ravel()].reshape(C, r1 - r0, NCND)
            s_ex[r0:r1] = np.einsum("cr,crk->rk",
                                    xnb.astype(np.float64)[:, r0:r1],
                                    gth.astype(np.float64), optimize=True)
        d_ex = x_sq[:, None] - 2.0 * s_ex + y_sq[cand]

        order = np.lexsort((cand, d_ex), axis=1)[:, :KK]
        top = np.take_along_axis(cand, order, axis=1)          # (NQ, 36)

        # saturation fallback: any group with >= 8 members in the top-36
        g36 = top // GS
        counts = np.zeros((NQ, G), np.int32)
        for gg in range(G):
            counts[:, gg] = (g36 == gg).sum(axis=1)
        bad = np.nonzero((counts >= 8).any(axis=1))[0]
        if len(bad):
            xnbad = xnb.astype(np.float64)[:, bad]
            s_full = xnbad.T @ ynb.astype(np.float64)
            d_full = x_sq[bad][:, None] - 2.0 * s_full + y_sq[None, :]
            idx_full = np.argsort(d_full, axis=1, kind="stable")[:, :KK]
            top[bad] = idx_full

        nn_idx[b, h * NQ:(h + 1) * NQ, :] = top

    center = np.broadcast_to(np.arange(N, dtype=np.int32)[None, :, None],
                             (B, N, K_OUT))
    edge = np.stack([np.ascontiguousarray(nn_idx[:, :, ::DIL]), center], axis=0)
    return edge.astype(np.int32)
